# revision 41
# baseline (speedup 1.0000x reference)
"""DirectPathAttenuationGNN Trainium2 kernel.

Strategy: data-parallel over graphs (512 graphs per core x 8 cores). The
graph topology is the fixed complete graph K9 (9 sensors, 72 directed
edges), so all gathers/scatters are per-graph-local and are expressed as
contiguous-slice / broadcast access patterns fed directly to the tensor
engine. Activations live transposed [H=128 partitions, tokens] in SBUF for
the whole network; only phys features stream in and per-edge logits stream
out. Matmuls run in float32r mode (1 cycle/row at N>=256).

Host side: phys edge-feature computation, weight folding (mean-aggregation
folded into node weights since deg==8), final sigmoid + pair-mean.
"""

import sys

if "/opt/trn_rl_repo" not in sys.path:
    sys.path.insert(0, "/opt/trn_rl_repo")

import numpy as np

B = 4096
S = 9
EPG = 72          # directed edges per graph
H = 128
L = 4
NCORES = 8
GC = B // NCORES  # graphs per core = 512
G = 256           # graphs per block
NBLK = GC // G    # 2
ET = EPG * G      # edge tokens per block = 18432
NT = S * G        # node tokens per block = 2304
TS = 512          # tile size (psum bank, fp32)
NTILE = ET // TS  # 36 edge tiles per block
EPS = np.float32(1e-8)

_prog_cache = {}


# ---------------------------------------------------------------------------
# host-side helpers
# ---------------------------------------------------------------------------

def _edge_struct():
    r_idx = np.repeat(np.arange(S), 8)              # [72] src node of edge e
    k_idx = np.tile(np.arange(8), S)
    c_idx = (r_idx + 1 + k_idx) % S                 # [72] dst node of edge e
    return r_idx, c_idx


def _build_phys(x_nodes, damage_locs):
    """phys [B, 72, 6] float32, device edge order, exact reference formulas."""
    xg = x_nodes.reshape(B, S, 2)
    r_idx, c_idx = _edge_struct()
    src = xg[:, r_idx, :]                           # [B,72,2]
    dst = xg[:, c_idx, :]
    dmg = damage_locs[:, None, :]                   # [B,1,2]

    vec = src - dst
    edge_len = np.sqrt(np.sum(vec * vec, -1) + EPS)
    d21 = dst - src
    l2 = np.clip(np.sum(d21 * d21, -1), EPS, None)
    t = np.clip(np.sum((dmg - src) * d21, -1) / l2, np.float32(0.0), np.float32(1.0))
    proj = src + t[..., None] * d21
    d_path = np.sqrt(np.sum((dmg - proj) ** 2, -1) + EPS)
    d_tx = np.sqrt(np.sum((src - dmg) ** 2, -1) + EPS)
    d_rx = np.sqrt(np.sum((dst - dmg) ** 2, -1) + EPS)
    phys = np.stack(
        [vec[..., 0], vec[..., 1], edge_len, d_path, d_tx, d_rx], axis=-1
    )
    return np.ascontiguousarray(phys.astype(np.float32))


# ---------------------------------------------------------------------------
# device program
# ---------------------------------------------------------------------------

def _build_program():
    from concourse import bacc, mybir, tile
    from contextlib import ExitStack

    f32 = mybir.dt.float32
    f32r = mybir.dt.float32r
    AF = mybir.ActivationFunctionType
    ALU = mybir.AluOpType

    nc = bacc.Bacc("TRN2", target_bir_lowering=False, debug=False)

    # ---- dram I/O
    xT_d = nc.dram_tensor("xT", [2, NBLK * NT], f32r, kind="ExternalInput")
    # phys features packed 4-up along partitions: rows 32q+f hold feature f of
    # edge tile 4m+q (for the row-packed K=6 encoder matmuls)
    physT_d = nc.dram_tensor("physT", [H, NBLK * ET // 4], f32r, kind="ExternalInput")
    # packed weights: per layer [w1c | w1a | w1b | w2 | wna | wnb | wn2]
    wl_d = nc.dram_tensor("wl", [H, L * 7 * H], f32r, kind="ExternalInput")
    # [encew2 | ident | decw1 | decw2b]
    wbig_d = nc.dram_tensor("wbig", [H, 2 * H + 64 + 2], f32r, kind="ExternalInput")
    # [encew1 replicated at partition bases 0/32/64/96 | encnw (2 rows)]
    encsm_d = nc.dram_tensor("encsm", [H, 2 * H], f32r, kind="ExternalInput")
    # biases: eb1[0:4] eb2[4:8] nb1[8:12] nb2[12:16] encnb[16] enceb1[17]
    #         enceb2[18] decb1x2[19]
    bp_d = nc.dram_tensor("bp", [H, 20], f32, kind="ExternalInput")
    z2_d = nc.dram_tensor("z2", [1, NBLK * ET], f32, kind="ExternalOutput")

    GSZ = 3                      # edge tiles per emission group
    NGRP = NTILE // GSZ          # 12

    with tile.TileContext(nc) as tc:
        with ExitStack() as ctx:
            wpool = ctx.enter_context(tc.tile_pool(name="w", bufs=1))
            sb = ctx.enter_context(tc.tile_pool(name="sb", bufs=1))
            ps = ctx.enter_context(tc.tile_pool(name="ps", bufs=1, space="PSUM"))

            # DMA order matters: encoder inputs first so compute starts
            # immediately; per-layer weight packs are emitted lazily at first
            # use so they queue behind only what precedes them.
            encsm = wpool.tile([H, 2 * H], f32r, name="encsm", tag="encsm")
            nc.sync.dma_start(encsm[:], encsm_d.ap())
            xT = sb.tile([2, NBLK * NT], f32r, name="xT_s", tag="xT_s")
            # chunked so the first node-encoder matmul isn't gated on the
            # whole transfer
            for xo in range(0, NBLK * NT, NT):
                nc.sync.dma_start(xT[:, xo:xo + NT], xT_d.ap()[:, xo:xo + NT])
            bp = wpool.tile([H, 20], f32, name="bp", tag="bp")
            nc.sync.dma_start(bp[:], bp_d.ap())
            wbig = wpool.tile([H, 2 * H + 64 + 2], f32r, name="wbig", tag="wbig")
            nc.sync.dma_start(wbig[:], wbig_d.ap())
            _prefetch_wl0 = True  # layer-0 weights queued right behind wbig

            encnw = encsm[0:2, H:2 * H]
            encew2 = wbig[:, 0:H]
            ident = wbig[:, H:2 * H]
            decw1 = wbig[:, 2 * H:2 * H + 64]
            decw2b = wbig[:, 2 * H + 64:2 * H + 66]
            eb1 = bp[:, 0:L]
            eb2 = bp[:, L:2 * L]
            nb1 = bp[:, 2 * L:3 * L]
            nb2 = bp[:, 3 * L:4 * L]
            encnb = bp[:, 16:17]
            enceb1 = bp[:, 17:18]
            enceb2 = bp[:, 18:19]
            decb1x2 = bp[:, 19:20]

            wl_tiles = {}

            def get_wl(l):
                """Layer-l packed weights, DMA'd on first use."""
                if l not in wl_tiles:
                    t = wpool.tile([H, 7 * H], f32r, name=f"wl{l}", tag=f"wl{l}")
                    nc.sync.dma_start(t[:], wl_d.ap()[:, l * 7 * H:(l + 1) * 7 * H])
                    wl_tiles[l] = t
                return wl_tiles[l]

            def wsl(l, k):
                return get_wl(l)[:, k * H:(k + 1) * H]
            # slice order: w1c=0, w1a=1, w1b=2, w2=3, wna=4, wnb=5, wn2=6

            get_wl(0)  # prefetch: layer 0 starts only ~6us into the kernel

            nt_tiles = [(0, 512), (512, 512), (1024, 512), (1536, 512), (2048, 256)]

            def node_phase_segments(blk, l, hn_src, hn_dst, wA, wB, w_2, bias1, bias2):
                """hn_dst = hn_src + MLP(hn_src, agg); reads hn_src only, writes
                hn_dst (ping-pong) so it runs fully parallel with the edge
                phase. Returned as small segments to interleave between edge
                groups so PE never waits on the intra-phase ACT/DVE chain."""
                state = {}

                def seg_s():
                    ps_s = ps.tile([H, TS], f32, name=f"pss{blk}_{l}", tag="psn", bufs=2)
                    for n in range(S):
                        nc.tensor.matmul(ps_s[:, :G], wB, hn_src[:, n * G:(n + 1) * G],
                                         start=(n == 0), stop=(n == S - 1))
                    s_t = sb.tile([H, G], f32r, name=f"st{blk}_{l}", tag="s_t", bufs=2)
                    nc.scalar.activation(s_t[:], ps_s[:, :G], AF.Identity, bias=bias1)
                    state["s_t"] = s_t
                    state["nm"] = []

                def seg_pre(tix):
                    s_t = state["s_t"]
                    for i in tix:
                        off, n = nt_tiles[i]
                        pn = ps.tile([H, TS], f32, name=f"pn{blk}_{l}_{i}", tag="psn", bufs=2)
                        nc.tensor.matmul(pn[:, :n], wA, hn_src[:, off:off + n],
                                         start=True, stop=False)
                        reps = n // G
                        if reps == 1:
                            nc.tensor.matmul(pn[:, :n], ident, s_t[:], start=False, stop=True)
                        else:
                            rhs_s = s_t[:].unsqueeze(1).to_broadcast((H, reps, G))
                            nc.tensor.matmul(pn[:, :n].rearrange("p (a b) -> p a b", a=reps),
                                             ident, rhs_s, start=False, stop=True)
                        nm = sb.tile([H, TS], f32r, name=f"nm{blk}_{l}_{i}", tag="nm", bufs=5)
                        nc.scalar.activation(nm[:, :n], pn[:, :n], AF.Relu, bias=0.0)
                        state["nm"].append((off, n, nm))

                def seg_post(tix, wrap=False):
                    for i in tix:
                        off, n, nm = state["nm"][i]
                        p2 = ps.tile([H, TS], f32, name=f"pn2{blk}_{l}_{i}", tag="psn", bufs=2)
                        nc.tensor.matmul(p2[:, :n], w_2, nm[:, :n])
                        nc.vector.scalar_tensor_tensor(hn_dst[:, off:off + n], p2[:, :n],
                                                       bias2, hn_src[:, off:off + n],
                                                       ALU.add, ALU.add)
                    if wrap:
                        nc.gpsimd.tensor_copy(hn_dst[:, S * G:17 * G], hn_dst[:, 0:8 * G])

                return [
                    seg_s,
                    lambda: seg_pre([0, 1]),
                    lambda: seg_pre([2, 3]),
                    lambda: seg_pre([4]),
                    lambda: seg_post([0, 1]),
                    lambda: seg_post([2, 3]),
                    lambda: seg_post([4], wrap=True),
                ]

            for blk in range(NBLK):
                he = sb.tile([H, ET], f32r, name=f"he{blk}", tag="he")
                hn = sb.tile([H, 17 * G], f32r, name=f"hn{blk}", tag="hn", bufs=2)

                # ---------------- node encoder: h_n = x @ enc_n_w + b
                for i, (off, n) in enumerate(nt_tiles):
                    pn = ps.tile([H, TS], f32, name=f"ne{blk}_{i}", tag="psn", bufs=2)
                    nc.tensor.matmul(pn[:, :n], encnw, xT[:, blk * NT + off: blk * NT + off + n])
                    nc.scalar.activation(hn[:, off:off + n], pn[:, :n], AF.Identity, bias=encnb)
                nc.vector.tensor_copy(hn[:, S * G:17 * G], hn[:, 0:8 * G])

                # ----- emission closures (pipelined groups) -----

                def enc_pre(grp):
                    """Edge encoder group: row-packed K=6 matmuls (4 tiles run
                    concurrently in 4 PE row strips) + relu evicts."""
                    ph = sb.tile([H, TS], f32r, name=f"ph{blk}_{grp}", tag="ph", bufs=3)
                    base = blk * (ET // 4) + grp * TS
                    nc.sync.dma_start(ph[:], physT_d.ap()[:, base:base + TS])
                    pres = []
                    for q in range(4):
                        t = 4 * grp + q
                        tag = "ps1" if q < 3 else "psn"
                        p1 = ps.tile([H, TS], f32, name=f"ee{blk}_{t}", tag=tag, bufs=3 if q < 3 else 2)
                        nc.tensor.matmul(p1[:], encsm[32 * q:32 * q + 6, 0:H],
                                         ph[32 * q:32 * q + 6, :],
                                         tile_position=(32 * q, 0))
                        pres.append((t, p1))
                    cur = []
                    for t, p1 in pres:
                        ze = sb.tile([H, TS], f32r, name=f"ze{blk}_{t}", tag="ze", bufs=6)
                        nc.scalar.activation(ze[:], p1[:], AF.Relu, bias=enceb1)
                        cur.append((t, ze))
                    return cur

                def enc_w2(items):
                    for i, (t, ze) in enumerate(items):
                        tag = "ps2" if i < 3 else "psn"
                        p2 = ps.tile([H, TS], f32, name=f"ee2{blk}_{t}", tag=tag, bufs=3 if i < 3 else 2)
                        nc.tensor.matmul(p2[:], encew2, ze[:])
                        sl = slice(t * TS, (t + 1) * TS)
                        nc.vector.tensor_scalar(he[:, sl], p2[:], enceb2, None, ALU.add)

                def edge_pre(l, grp, hn_cur):
                    p1s = []
                    for q in range(GSZ):
                        t = GSZ * grp + q
                        p1 = ps.tile([H, TS], f32, name=f"pe{blk}_{l}_{t}", tag="ps1", bufs=3)
                        p1s.append((t, p1))
                    for t, p1 in p1s:
                        nc.tensor.matmul(p1[:], wsl(l, 0), he[:, t * TS:(t + 1) * TS],
                                         start=True, stop=False)
                    for t, p1 in p1s:
                        r = t // 4
                        rhs_ta = hn_cur[:, r * G:(r + 1) * G].unsqueeze(1).to_broadcast((H, 2, G))
                        nc.tensor.matmul(p1[:].rearrange("p (a b) -> p a b", a=2),
                                         wsl(l, 1), rhs_ta, start=False, stop=False)
                    for t, p1 in p1s:
                        r, q4 = divmod(t, 4)
                        off = (r + 1 + 2 * q4) * G
                        nc.tensor.matmul(p1[:], wsl(l, 2), hn_cur[:, off:off + TS],
                                         start=False, stop=True)
                    cur = []
                    for t, p1 in p1s:
                        msg = sb.tile([H, TS], f32r, name=f"mg{blk}_{l}_{t}", tag="msg", bufs=6)
                        nc.scalar.activation(msg[:], p1[:], AF.Relu, bias=eb1[:, l:l + 1])
                        cur.append((t, msg))
                    return cur

                def edge_w2(l, items):
                    for t, msg in items:
                        p2 = ps.tile([H, TS], f32, name=f"pe2{blk}_{l}_{t}", tag="ps2", bufs=3)
                        nc.tensor.matmul(p2[:], wsl(l, 3), msg[:])
                        sl = slice(t * TS, (t + 1) * TS)
                        nc.vector.scalar_tensor_tensor(he[:, sl], p2[:], eb2[:, l:l + 1],
                                                       he[:, sl], ALU.add, ALU.add)

                def dec_pre(grp):
                    pr1 = []
                    for q in range(GSZ):
                        t = GSZ * grp + q
                        p1 = ps.tile([H, TS], f32, name=f"pd{blk}_{t}", tag="ps1", bufs=3)
                        nc.tensor.matmul(p1[0:64, :], decw1, he[:, t * TS:(t + 1) * TS])
                        pr1.append((t, p1))
                    cur = []
                    for t, p1 in pr1:
                        z = sb.tile([64, TS], f32r, name=f"z{blk}_{t}", tag="z", bufs=6)
                        nc.scalar.activation(z[:], p1[0:64, :], AF.Relu, bias=decb1x2[0:64, :])
                        cur.append((t, z))
                    return cur

                def dec_tail(items):
                    for i, (t, z) in enumerate(items):
                        tag = "ps2" if i < 3 else "psn"
                        p2 = ps.tile([1, TS], f32, name=f"pd2{blk}_{t}", tag=tag, bufs=3 if i < 3 else 2)
                        nc.tensor.matmul(p2[:], decw2b[0:64, 0:1], z[:])
                        zo = sb.tile([1, TS], f32, name=f"zo{blk}_{t}", tag="zo", bufs=5)
                        nc.vector.tensor_copy(zo[:], p2[:])
                        off = blk * ET + t * TS
                        nc.sync.dma_start(z2_d.ap()[:, off:off + TS], zo[:])

                # ---------------- encoder + layer 0, interleaved.
                # dep math: layer-0 group k reads he tiles 3k..3k+2, which the
                # encoder W2 lag has evicted by combined step k+2.
                hn1 = sb.tile([H, 17 * G], f32r, name=f"hn{blk}_0", tag="hn", bufs=2)
                segs0 = node_phase_segments(blk, 0, hn, hn1,
                                            wsl(0, 4), wsl(0, 5), wsl(0, 6),
                                            nb1[:, 0:1], nb2[:, 0:1])
                encprev, l0prev = [], []
                for step in range(NGRP + 3):
                    enccur = enc_pre(step) if step < 9 else []
                    enc_w2(encprev)
                    encprev = enccur
                    k = step - 2
                    l0cur = edge_pre(0, k, hn) if 0 <= k < NGRP else []
                    edge_w2(0, l0prev)
                    l0prev = l0cur
                    if 1 <= k <= len(segs0):
                        segs0[k - 1]()
                hn_cur = hn1

                # ---------------- layers 1..2 (node segments interleaved)
                for l in (1, 2):
                    hn_next = sb.tile([H, 17 * G], f32r, name=f"hn{blk}_{l}", tag="hn", bufs=2)
                    segs = node_phase_segments(blk, l, hn_cur, hn_next,
                                               wsl(l, 4), wsl(l, 5), wsl(l, 6),
                                               nb1[:, l:l + 1], nb2[:, l:l + 1])
                    prev = []
                    for grp in range(NGRP + 1):
                        cur = edge_pre(l, grp, hn_cur) if grp < NGRP else []
                        edge_w2(l, prev)
                        if 1 <= grp <= len(segs):
                            segs[grp - 1]()
                        prev = cur
                    hn_cur = hn_next

                # ---------------- layer 3 + decoder, interleaved.
                # layer 3 has no node update (its output would be unused).
                # dep math: decoder group k reads he tiles 3k..3k+2, final
                # after layer-3's W2/stt of group k at combined step k+1.
                l3prev, decprev = [], []
                for step in range(NGRP + 3):
                    l3cur = edge_pre(3, step, hn_cur) if step < NGRP else []
                    edge_w2(3, l3prev)
                    l3prev = l3cur
                    k = step - 2
                    deccur = dec_pre(k) if 0 <= k < NGRP else []
                    dec_tail(decprev)
                    decprev = deccur

    nc.compile()
    return nc


def _get_program():
    if "nc" not in _prog_cache:
        _prog_cache["nc"] = _build_program()
    return _prog_cache["nc"]


# ---------------------------------------------------------------------------
# kernel entry
# ---------------------------------------------------------------------------

def kernel(x_nodes, damage_locs,
           enc_n_w, enc_n_b, enc_e_w1, enc_e_b1, enc_e_w2, enc_e_b2,
           edge_w1, edge_b1, edge_w2, edge_b2,
           node_w1, node_b1, node_w2, node_b2,
           dec_w1, dec_b1, dec_w2, dec_b2,
           edge_index, node_batch):
    import os
    from concourse.bass_utils import run_bass_kernel_spmd

    f32 = np.float32
    x_nodes = np.asarray(x_nodes, f32)
    damage_locs = np.asarray(damage_locs, f32)

    # ---- host precompute
    phys = _build_phys(x_nodes, damage_locs)                  # [B,72,6]

    def cat(ws):
        return np.ascontiguousarray(np.concatenate(ws, axis=0).astype(f32))

    edge_w1 = np.asarray(edge_w1, f32)
    node_w1 = np.asarray(node_w1, f32)
    w1a = cat([edge_w1[l, 0:H, :] for l in range(L)])
    w1b = cat([edge_w1[l, H:2 * H, :] for l in range(L)])
    w1c = cat([edge_w1[l, 2 * H:3 * H, :] for l in range(L)])
    w2 = cat([np.asarray(edge_w2, f32)[l] for l in range(L)])
    wna = cat([node_w1[l, 0:H, :] - node_w1[l, H:2 * H, :] / f32(8.0) for l in range(L)])
    wnb = cat([node_w1[l, H:2 * H, :] / f32(8.0) for l in range(L)])
    wn2 = cat([np.asarray(node_w2, f32)[l] for l in range(L)])
    eb1 = np.ascontiguousarray(np.asarray(edge_b1, f32).T)    # [H,L]
    eb2 = np.ascontiguousarray(np.asarray(edge_b2, f32).T)
    nb1 = np.ascontiguousarray(np.asarray(node_b1, f32).T)
    nb2 = np.ascontiguousarray(np.asarray(node_b2, f32).T)

    dec_w2 = np.asarray(dec_w2, f32)                          # [64, 1]
    decw2b = np.zeros((H, 2), f32)
    decw2b[0:64, 0] = dec_w2[:, 0]
    decw2b[64:128, 1] = dec_w2[:, 0]
    decb1x2 = np.concatenate([np.asarray(dec_b1, f32)] * 2)[:, None]

    # packed weights: per layer [w1c | w1a | w1b | w2 | wna | wnb | wn2]
    wl = np.concatenate(
        [np.concatenate([w1c[l * H:(l + 1) * H], w1a[l * H:(l + 1) * H],
                         w1b[l * H:(l + 1) * H], w2[l * H:(l + 1) * H],
                         wna[l * H:(l + 1) * H], wnb[l * H:(l + 1) * H],
                         wn2[l * H:(l + 1) * H]], axis=1) for l in range(L)],
        axis=1)                                               # [H, L*7*H]
    decw1_a = np.asarray(dec_w1, f32)
    wbig = np.concatenate(
        [np.asarray(enc_e_w2, f32), np.eye(H, dtype=f32), decw1_a, decw2b], axis=1)
    encsm = np.zeros((H, 2 * H), f32)
    for q in range(4):
        encsm[32 * q:32 * q + 6, 0:H] = np.asarray(enc_e_w1, f32)
    encsm[0:2, H:2 * H] = np.asarray(enc_n_w, f32)
    bpk = np.zeros((H, 20), f32)
    bpk[:, 0:L] = eb1
    bpk[:, L:2 * L] = eb2
    bpk[:, 2 * L:3 * L] = nb1
    bpk[:, 3 * L:4 * L] = nb2
    bpk[:, 16] = np.asarray(enc_n_b, f32)
    bpk[:, 17] = np.asarray(enc_e_b1, f32)
    bpk[:, 18] = np.asarray(enc_e_b2, f32)
    bpk[:, 19] = decb1x2[:, 0]

    shared = dict(
        wl=np.ascontiguousarray(wl),
        wbig=np.ascontiguousarray(wbig),
        encsm=np.ascontiguousarray(encsm),
        bp=np.ascontiguousarray(bpk),
    )

    xg = x_nodes.reshape(B, S, 2)
    in_maps = []
    for c in range(NCORES):
        gsl = slice(c * GC, (c + 1) * GC)
        # xT: [2, blk*NT + n*G + g]
        xc = xg[gsl].reshape(NBLK, G, S, 2).transpose(3, 0, 2, 1).reshape(2, -1)
        # physT: [6, blk*ET + e*G + g] then 4-up row packing:
        # physT4[32q+f, blk*ET/4 + m*TS + j] = pc[f, blk, tile 4m+q, token j]
        pc = phys[gsl].reshape(NBLK, G, EPG, 6).transpose(3, 0, 2, 1).reshape(6, -1)
        pc5 = pc.reshape(6, NBLK, ET // (4 * TS), 4, TS)
        p4 = np.zeros((H, NBLK * ET // 4), f32)
        p4v = p4.reshape(H, NBLK, ET // (4 * TS), TS)
        for q in range(4):
            p4v[32 * q:32 * q + 6] = pc5[:, :, :, q, :]
        m = dict(shared)
        m["xT"] = np.ascontiguousarray(xc)
        m["physT"] = np.ascontiguousarray(p4)
        in_maps.append(m)

    nc = _get_program()
    trace = bool(int(os.environ.get("KERNEL_TRACE", "0")))
    res = None
    for attempt in range(3):
        try:
            res = run_bass_kernel_spmd(nc, in_maps, core_ids=list(range(NCORES)),
                                       trace=trace)
            break
        except Exception:
            if attempt == 2:
                raise
    _prog_cache["last_results"] = res

    # ---- host postprocess: sigmoid + pair mean
    z2 = np.empty((B, EPG), f32)
    for c in range(NCORES):
        zc = res.results[c]["z2"].reshape(NBLK, EPG, G).transpose(0, 2, 1).reshape(GC, EPG)
        z2[c * GC:(c + 1) * GC] = zc

    logits = z2 + np.asarray(dec_b2, f32)[0]
    sig = f32(1.0) / (f32(1.0) + np.exp(-logits))

    pairs = [(i, j) for i in range(S) for j in range(i + 1, S)]
    out = np.empty((B, len(pairs)), f32)
    for p, (i, j) in enumerate(pairs):
        a = i * 8 + (j - i - 1)
        bidx = j * 8 + (8 - (j - i))
        out[:, p] = f32(0.5) * (sig[:, a] + sig[:, bidx])
    return out
